# revision 10
# baseline (speedup 1.0000x reference)
"""F1-score (macro) kernel for Trainium2, 8 NeuronCores.

Layout: per core (data-parallel over rows), rows are partition-contiguous:
local row = p*J + j  (p in [0,128), j in [0,976)), 72-row tail handled flat.
Tiles of TK=61 j-columns: xh [128, 61, 128] bf16, cast f32->bf16 during the
SWDGE DMA itself (HBM reads stay f32 = the memory roofline; SBUF and all
compute go 16-bit for the DVE 2x/4x perf modes).

Per tile:
  - DVE: oht[:,j,:] = (iota == t[:, j])       tensor_scalar is_equal, 4x bf16
  - DVE: rowmax via 4-stage tensor_tensor max tree (2x bf16) + small reduce
  - DVE: ohp[:,j,:] = (xh == mh)  for j in DVE chunks (is_equal, 4x)
  - ACT: ohp[:,j,:] = sign(mh - xh) (= anti) for j in ACT chunks
  - PE : acc_p += oht_j^T @ ohp_j (DVE chunks), acc_a += oht_j^T @ anti_j
Host: cm = sum_cores [acc_p + (support_act - acc_a)]; macro-F1 epilogue.

bf16 tie semantics (multi-hot on exact bf16 ties) verified on the harness
data: rel err 7.7e-4 << 2e-2.
"""

import sys
import time

if "/opt/trn_rl_repo" not in sys.path:
    sys.path.insert(0, "/opt/trn_rl_repo")

import numpy as np

import concourse.bacc as bacc
import concourse.mybir as mybir
import concourse.tile as tile
from concourse import bass_utils

C = 128
N = 1_000_000
NCORES = 8
R = N // NCORES          # 125000 rows per core
J = 976                  # j-columns per partition (128*976 = 124928 rows)
TK = 61                  # j-columns per tile
NT = J // TK             # 16 tiles
TAIL = R - 128 * J       # 72 leftover rows
EPS = 1e-12

N_ACT = 52               # ohp chunks per tile computed on ACT (Sign/anti form)
ACT_SET = tuple(range(TK - N_ACT, TK))   # j-locals assigned to ACT
GS_OHT_SET = ()          # oht chunks on GpSimd: measured net-negative (DMA + DVE interference)

_CACHE = {}


def _build():
    f32 = mybir.dt.float32
    bf16 = mybir.dt.bfloat16
    Alu = mybir.AluOpType
    Act = mybir.ActivationFunctionType

    nc = bacc.Bacc("TRN2", target_bir_lowering=False, debug=False,
                   num_devices=NCORES)
    yp = nc.dram_tensor("yp", [R, C], f32, kind="ExternalInput")
    yt = nc.dram_tensor("yt", [R], f32, kind="ExternalInput")
    cm = nc.dram_tensor("cm", [C, 2 * C], f32, kind="ExternalOutput")

    yp_grid = yp.ap()[0 : 128 * J, :].rearrange("(p j) c -> p j c", p=128)
    yt_grid = yt.ap()[0 : 128 * J].rearrange("(p j) -> p j", p=128)

    with tile.TileContext(nc) as tc:
        with (
            tc.tile_pool(name="const", bufs=1) as cpool,
            tc.tile_pool(name="xin", bufs=3) as xpool,
            tc.tile_pool(name="tree", bufs=1) as tpool,
            tc.tile_pool(name="oh", bufs=3) as ohpool,
            tc.tile_pool(name="small", bufs=3) as spool,
            tc.tile_pool(name="psum", bufs=1, space="PSUM") as psum,
        ):
            iota_i = cpool.tile([128, C], mybir.dt.int32)
            nc.gpsimd.iota(iota_i[:], pattern=[[1, C]], base=0,
                           channel_multiplier=0)
            iota_h = cpool.tile([128, C], bf16)
            nc.vector.tensor_copy(iota_h[:], iota_i[:])

            # whole y_true for the grid (f32: tensor_scalar scalars must be f32)
            t_all = cpool.tile([128, J], f32)
            nc.sync.dma_start(t_all[:], yt_grid)
            t_tail = cpool.tile([TAIL, 1], f32)
            nc.sync.dma_start(
                t_tail[:], yt.ap()[128 * J : R].rearrange("(p k) -> p k", k=1)
            )

            acc_p = psum.tile([C, C], f32)
            acc_a = psum.tile([C, C], f32)
            state = {"p": False, "a": False}
            n_a_total = NT * N_ACT

            def emit_tile(i):
                j0 = i * TK
                xh = xpool.tile([128, TK, C], bf16, tag="xh")
                nc.gpsimd.dma_start(xh[:], yp_grid[:, j0 : j0 + TK, :])

                oht = ohpool.tile([128, TK, C], bf16, tag="oht")
                for j in range(TK):
                    eng = nc.gpsimd if j in GS_OHT_SET else nc.vector
                    eng.tensor_scalar(
                        oht[:, j, :], iota_h[:], t_all[:, j0 + j : j0 + j + 1],
                        None, op0=Alu.is_equal,
                    )

                # rowmax tree: 64 -> 32 -> 16 -> 8 then reduce
                m1 = tpool.tile([128, TK, 64], bf16, tag="m1")
                nc.vector.tensor_tensor(
                    m1[:], xh[:, :, 0:64], xh[:, :, 64:128], op=Alu.max
                )
                m2 = tpool.tile([128, TK, 32], bf16, tag="m2")
                nc.vector.tensor_tensor(
                    m2[:], m1[:, :, 0:32], m1[:, :, 32:64], op=Alu.max
                )
                m3 = tpool.tile([128, TK, 16], bf16, tag="m3")
                nc.vector.tensor_tensor(
                    m3[:], m2[:, :, 0:16], m2[:, :, 16:32], op=Alu.max
                )
                m4 = tpool.tile([128, TK, 8], bf16, tag="m4")
                nc.vector.tensor_tensor(
                    m4[:], m3[:, :, 0:8], m3[:, :, 8:16], op=Alu.max
                )
                m5 = tpool.tile([128, TK, 4], bf16, tag="m5")
                nc.vector.tensor_tensor(
                    m5[:], m4[:, :, 0:4], m4[:, :, 4:8], op=Alu.max
                )
                m6 = tpool.tile([128, TK, 2], bf16, tag="m6")
                nc.vector.tensor_tensor(
                    m6[:], m5[:, :, 0:2], m5[:, :, 2:4], op=Alu.max
                )
                mh = spool.tile([128, TK], f32, tag="mh")
                nc.vector.tensor_tensor(
                    mh[:, :, None], m6[:, :, 0:1], m6[:, :, 1:2], op=Alu.max
                )

                ohp = ohpool.tile([128, TK, C], bf16, tag="ohp")
                for j in range(TK):
                    if j in ACT_SET:
                        nc.scalar.activation(
                            ohp[:, j, :], xh[:, j, :], Act.Sign,
                            bias=mh[:, j : j + 1], scale=-1.0,
                        )
                        acc, key = acc_a, "a"
                    else:
                        nc.vector.tensor_scalar(
                            ohp[:, j, :], xh[:, j, :], mh[:, j : j + 1],
                            None, op0=Alu.is_equal,
                        )
                        acc, key = acc_p, "p"
                    nc.tensor.matmul(
                        acc[:], oht[:, j, :], ohp[:, j, :],
                        start=not state[key], stop=False,
                    )
                    state[key] = True

            for i in range(NT):
                emit_tile(i)

            # mark end of acc_a accumulation with a zero-contribution matmul?
            # Instead: reuse last ACT matmul as stop by emitting tail first is
            # complex; simply do a final stop matmul on acc_a with zero rows is
            # not possible -- use stop on a redundant matmul of zeros.
            # Simpler: tail goes to acc_p with stop=True, and acc_a gets its
            # stop flag via a final 1-row matmul of zeros.

            # tail rows (72)
            xt = xpool.tile([TAIL, 1, C], bf16, tag="xtail")
            nc.gpsimd.dma_start(
                xt[:],
                yp.ap()[128 * J : R, :].rearrange("(p k) c -> p k c", k=1),
            )
            mh_t = spool.tile([TAIL, 1], f32, tag="mhtail")
            nc.vector.tensor_reduce(
                mh_t[:], xt[:], axis=mybir.AxisListType.X, op=Alu.max
            )
            ohp_t = ohpool.tile([TAIL, C], bf16, tag="ohptail")
            oht_t = ohpool.tile([TAIL, C], bf16, tag="ohttail")
            nc.vector.tensor_scalar(
                ohp_t[:], xt[:, 0, :], mh_t[:], None, op0=Alu.is_equal
            )
            nc.vector.tensor_scalar(
                oht_t[:], iota_h[:TAIL, :], t_tail[:], None, op0=Alu.is_equal
            )
            nc.tensor.matmul(
                acc_p[:], oht_t[:], ohp_t[:], start=False, stop=True
            )
            # close acc_a accumulation: repeat the last tail matmul shape into
            # acc_a with zero operands? Use oht_t row0 x zero vector instead.
            zrow = cpool.tile([1, C], bf16)
            nc.vector.memset(zrow[:], 0.0)
            nc.tensor.matmul(
                acc_a[:], zrow[:], zrow[:], start=False, stop=True
            )

            out_sb = spool.tile([C, 2 * C], f32, tag="out")
            nc.scalar.copy(out_sb[:, 0:C], acc_p[:])
            nc.scalar.copy(out_sb[:, C : 2 * C], acc_a[:])
            nc.sync.dma_start(cm.ap()[:], out_sb[:])

    nc.compile()
    return nc


def _get_nc():
    if "nc" not in _CACHE:
        _CACHE["nc"] = _build()
    return _CACHE["nc"]


def _act_row_mask():
    """Bool mask over local rows [0, R): rows whose chunk went to ACT."""
    jl = np.arange(J) % TK
    jmask = np.isin(jl, np.asarray(ACT_SET))
    mask = np.zeros(R, dtype=bool)
    mask[: 128 * J] = np.broadcast_to(jmask, (128, J)).ravel()
    return mask


def _run(y_pred, y_true, trace=False):
    nc = _get_nc()
    y_pred = np.ascontiguousarray(np.asarray(y_pred, dtype=np.float32))
    yt_i = np.asarray(y_true).astype(np.int64)
    yt_f = yt_i.astype(np.float32)
    in_maps = [
        {
            "yp": y_pred[c * R : (c + 1) * R],
            "yt": np.ascontiguousarray(yt_f[c * R : (c + 1) * R]),
        }
        for c in range(NCORES)
    ]
    res = None
    for attempt in range(3):
        try:
            res = bass_utils.run_bass_kernel_spmd(
                nc, in_maps, core_ids=list(range(NCORES)), trace=trace
            )
            break
        except Exception:
            if attempt == 2:
                raise
            time.sleep(2.0)
    amask = _act_row_mask()
    cm_total = np.zeros((C, C), dtype=np.float64)
    for c, r in enumerate(res.results):
        out = r["cm"].astype(np.float64)
        acc_p, acc_a = out[:, 0:C], out[:, C : 2 * C]
        yt_core = yt_i[c * R : (c + 1) * R]
        support_act = np.bincount(yt_core[amask], minlength=C).astype(
            np.float64
        )
        cm_total += acc_p + (support_act[:, None] - acc_a)
    diag = np.diagonal(cm_total)
    precision = diag / (cm_total.sum(axis=1) + EPS)
    recall = diag / (cm_total.sum(axis=0) + EPS)
    f1 = 2.0 * precision * recall / (precision + recall + EPS)
    return np.float32(f1.mean()), res


def kernel(y_pred, y_true):
    out, _ = _run(y_pred, y_true, trace=False)
    return out


# revision 12
# speedup vs baseline: 1.5225x; 1.5225x over previous
"""F1-score (macro) kernel for Trainium2, 8 NeuronCores.

Key trick: the confusion matrix is invariant under row permutation, so the
host reorders rows so that most device chunks [128 rows, 128 classes] hold
exactly one row of each true class at partition p = class.  For those chunks
the true-label one-hot stationary is the IDENTITY (a constant in SBUF) -- no
per-chunk oht build at all.  Rows that don't fit the per-class quota go to 32
"regular" chunks with an on-device oht build; missing slots become pad rows
(y_pred = e0, so pred=0) whose exact contribution is subtracted host-side.

Device layout per core: 125056 rows = 128 partitions x 977 j-columns,
row(p, j) = p*977 + j.  16 tiles (15x61 + 1x62 j-cols).  Per tile:
  - SWDGE DMA casts y_pred f32 -> bf16 on the fly (HBM reads stay f32)
  - DVE: rowmax via 7-stage tensor_tensor max tree (2x bf16 mode)
  - ohp[:,j,:]: DVE is_equal(xh, mh) for ~33 chunks -> acc_p,
                ACT sign(mh - xh) (anti form) for 28 chunks -> acc_a
  - PE: acc += stationary^T @ ohp, stationary = identity or built oht
Host: cm = sum_cores [acc_p + (support_act - acc_a)] - pad corrections.

bf16 tie semantics (multi-hot on exact bf16 ties) verified on harness data:
rel err 7.7e-4 << 2e-2.
"""

import sys
import time

if "/opt/trn_rl_repo" not in sys.path:
    sys.path.insert(0, "/opt/trn_rl_repo")

import numpy as np

import concourse.bacc as bacc
import concourse.mybir as mybir
import concourse.tile as tile
from concourse import bass_utils

C = 128
N = 1_000_000
NCORES = 8
R = N // NCORES          # 125000 real rows per core
J = 977                  # j-columns per partition
RD = 128 * J             # 125056 device rows per core
TKS = [61] * 15 + [62]   # j-columns per tile
REG_LOCALS = (29, 60)    # j-locals per tile with on-device oht build
ACT_LOCALS = frozenset(range(33, 61))  # ohp chunks computed on ACT
K_ID = J - 16 * len(REG_LOCALS)        # 945 identity chunks per core
EPS = 1e-12

_J0S = np.cumsum([0] + TKS[:-1]).tolist()
REG_JS = sorted(j0 + l for j0 in _J0S for l in REG_LOCALS)
IDENT_JS = sorted(set(range(J)) - set(REG_JS))
ACT_JS = sorted(
    j0 + l for i, j0 in enumerate(_J0S) for l in range(TKS[i]) if l in ACT_LOCALS
)

_CACHE = {}


def _build():
    f32 = mybir.dt.float32
    bf16 = mybir.dt.bfloat16
    Alu = mybir.AluOpType
    Act = mybir.ActivationFunctionType

    nc = bacc.Bacc("TRN2", target_bir_lowering=False, debug=False,
                   num_devices=NCORES)
    yp = nc.dram_tensor("yp", [RD, C], f32, kind="ExternalInput")
    yt = nc.dram_tensor("yt", [128 * len(REG_JS)], f32, kind="ExternalInput")
    cm = nc.dram_tensor("cm", [C, 2 * C], f32, kind="ExternalOutput")

    grid = yp.ap().rearrange("(p j) c -> p j c", p=128)

    with tile.TileContext(nc) as tc:
        with (
            tc.tile_pool(name="const", bufs=1) as cpool,
            tc.tile_pool(name="xin", bufs=3) as xpool,
            tc.tile_pool(name="tree", bufs=1) as tpool,
            tc.tile_pool(name="oh", bufs=3) as ohpool,
            tc.tile_pool(name="small", bufs=3) as spool,
            tc.tile_pool(name="psum", bufs=1, space="PSUM") as psum,
        ):
            iota_i = cpool.tile([128, C], mybir.dt.int32)
            nc.gpsimd.iota(iota_i[:], pattern=[[1, C]], base=0,
                           channel_multiplier=0)
            iota_h = cpool.tile([128, C], bf16)
            nc.vector.tensor_copy(iota_h[:], iota_i[:])
            # per-partition index column p -> identity = (iota == p)
            pcol_i = cpool.tile([128, 1], mybir.dt.int32)
            nc.gpsimd.iota(pcol_i[:], pattern=[[0, 1]], base=0,
                           channel_multiplier=1)
            pcol = cpool.tile([128, 1], f32)
            nc.vector.tensor_copy(pcol[:], pcol_i[:])
            ident = cpool.tile([128, C], bf16)
            nc.vector.tensor_scalar(
                ident[:], iota_h[:], pcol[:], None, op0=Alu.is_equal
            )

            # true labels for the regular chunks only: [128, 32]
            t_reg = cpool.tile([128, len(REG_JS)], f32)
            nc.sync.dma_start(
                t_reg[:], yt.ap().rearrange("(p k) -> p k", p=128)
            )
            reg_idx = {j: k for k, j in enumerate(REG_JS)}

            acc_p = psum.tile([C, C], f32)
            acc_a = psum.tile([C, C], f32)
            state = {"p": False, "a": False}

            def emit_tile(i):
                j0, tk = _J0S[i], TKS[i]
                xh = xpool.tile([128, tk, C], bf16, tag="xh")
                nc.gpsimd.dma_start(xh[:], grid[:, j0 : j0 + tk, :])

                # oht builds for this tile's regular chunks (DMA-independent)
                ohts = {}
                for l in REG_LOCALS:
                    if l >= tk:
                        continue
                    o = ohpool.tile([128, C], bf16, tag=f"oht{l}")
                    k = reg_idx[j0 + l]
                    nc.vector.tensor_scalar(
                        o[:], iota_h[:], t_reg[:, k : k + 1], None,
                        op0=Alu.is_equal,
                    )
                    ohts[l] = o

                # rowmax tree: 64->32->16->8->4->2->1 (bf16 2x TT stages)
                m1 = tpool.tile([128, tk, 64], bf16, tag="m1")
                nc.vector.tensor_tensor(
                    m1[:], xh[:, :, 0:64], xh[:, :, 64:128], op=Alu.max
                )
                m2 = tpool.tile([128, tk, 32], bf16, tag="m2")
                nc.vector.tensor_tensor(
                    m2[:], m1[:, :, 0:32], m1[:, :, 32:64], op=Alu.max
                )
                m3 = tpool.tile([128, tk, 16], bf16, tag="m3")
                nc.vector.tensor_tensor(
                    m3[:], m2[:, :, 0:16], m2[:, :, 16:32], op=Alu.max
                )
                m4 = tpool.tile([128, tk, 8], bf16, tag="m4")
                nc.vector.tensor_tensor(
                    m4[:], m3[:, :, 0:8], m3[:, :, 8:16], op=Alu.max
                )
                m5 = tpool.tile([128, tk, 4], bf16, tag="m5")
                nc.vector.tensor_tensor(
                    m5[:], m4[:, :, 0:4], m4[:, :, 4:8], op=Alu.max
                )
                m6 = tpool.tile([128, tk, 2], bf16, tag="m6")
                nc.vector.tensor_tensor(
                    m6[:], m5[:, :, 0:2], m5[:, :, 2:4], op=Alu.max
                )
                mh = spool.tile([128, tk], f32, tag="mh")
                nc.vector.tensor_tensor(
                    mh[:, :, None], m6[:, :, 0:1], m6[:, :, 1:2], op=Alu.max
                )

                ohp = ohpool.tile([128, tk, C], bf16, tag="ohp")
                for l in range(tk):
                    stat = ohts.get(l, ident)
                    if l in ACT_LOCALS:
                        nc.scalar.activation(
                            ohp[:, l, :], xh[:, l, :], Act.Sign,
                            bias=mh[:, l : l + 1], scale=-1.0,
                        )
                        acc, key = acc_a, "a"
                    else:
                        nc.vector.tensor_scalar(
                            ohp[:, l, :], xh[:, l, :], mh[:, l : l + 1],
                            None, op0=Alu.is_equal,
                        )
                        acc, key = acc_p, "p"
                    nc.tensor.matmul(
                        acc[:], stat[:], ohp[:, l, :],
                        start=not state[key], stop=False,
                    )
                    state[key] = True

            for i in range(len(TKS)):
                emit_tile(i)

            # close both accumulation groups
            zrow = cpool.tile([1, C], bf16)
            nc.vector.memset(zrow[:], 0.0)
            nc.tensor.matmul(acc_p[:], zrow[:], zrow[:], start=False, stop=True)
            nc.tensor.matmul(acc_a[:], zrow[:], zrow[:], start=False, stop=True)

            out_sb = spool.tile([C, 2 * C], f32, tag="out")
            nc.scalar.copy(out_sb[:, 0:C], acc_p[:])
            nc.scalar.copy(out_sb[:, C : 2 * C], acc_a[:])
            nc.sync.dma_start(cm.ap()[:], out_sb[:])

    nc.compile()
    return nc


def _get_nc():
    if "nc" not in _CACHE:
        _CACHE["nc"] = _build()
    return _CACHE["nc"]


def _layout(yt_i):
    """Assign global rows to device slots.

    Returns per-core: idx [128, J] (global row id, -1 => pad),
    pad_class [128, J] (true class of pad slots, valid where idx < 0),
    and the per-core device true-class grid tcls [128, J].
    """
    idxs, tclss = [], []
    rows_by_class = [np.flatnonzero(yt_i == t) for t in range(C)]
    surplus = []
    per_core_ident = [dict() for _ in range(NCORES)]
    for t in range(C):
        rows_t = rows_by_class[t]
        for c in range(NCORES):
            seg = rows_t[c * K_ID : (c + 1) * K_ID]
            per_core_ident[c][t] = seg
        surplus.append(rows_t[NCORES * K_ID :])
    pool = (
        np.concatenate(surplus)
        if surplus
        else np.zeros(0, dtype=np.int64)
    )
    nreg = 128 * len(REG_JS)
    parts = np.array_split(pool, NCORES)
    ident_js = np.asarray(IDENT_JS)
    reg_js = np.asarray(REG_JS)
    for c in range(NCORES):
        idx = np.full((128, J), -1, dtype=np.int64)
        tcls = np.zeros((128, J), dtype=np.int64)
        for t in range(C):
            seg = per_core_ident[c][t]
            idx[t, ident_js[: len(seg)]] = seg
            tcls[t, ident_js] = t  # pads in ident region keep class t
        part = parts[c]
        take = min(len(part), nreg)
        # fill reg slots p-major: slot k -> (p = k % 128, j = reg_js[k // 128])
        ks = np.arange(take)
        idx[ks % 128, reg_js[ks // 128]] = part[:take]
        tcls[ks % 128, reg_js[ks // 128]] = yt_i[part[:take]]
        # remaining reg slots stay pads with class 0 (tcls already 0)
        idxs.append(idx)
        tclss.append(tcls)
    return idxs, tclss


def _run(y_pred, y_true, trace=False):
    nc = _get_nc()
    y_pred = np.ascontiguousarray(np.asarray(y_pred, dtype=np.float32))
    yt_i = np.asarray(y_true).astype(np.int64)
    idxs, tclss = _layout(yt_i)

    pad_row = np.zeros(C, dtype=np.float32)
    pad_row[0] = 1.0  # pred = 0 for pad rows

    in_maps = []
    supports_act = []
    pad_corr = np.zeros(C, dtype=np.float64)  # pads predict 0: cm[t,0] -= corr
    for c in range(NCORES):
        idx = idxs[c]
        tcls = tclss[c]
        flat = idx.ravel()
        pads = flat < 0
        yp_dev = y_pred[np.where(pads, 0, flat)]
        if pads.any():
            yp_dev[pads] = pad_row
        yp_dev = np.ascontiguousarray(yp_dev)
        t_reg = np.ascontiguousarray(
            tcls[:, REG_JS].astype(np.float32)
        ).ravel()
        in_maps.append({"yp": yp_dev, "yt": t_reg})
        supports_act.append(
            np.bincount(tcls[:, ACT_JS].ravel(), minlength=C).astype(
                np.float64
            )
        )
        pad_corr += np.bincount(
            tcls.ravel()[pads], minlength=C
        ).astype(np.float64)

    res = None
    for attempt in range(3):
        try:
            res = bass_utils.run_bass_kernel_spmd(
                nc, in_maps, core_ids=list(range(NCORES)), trace=trace
            )
            break
        except Exception:
            if attempt == 2:
                raise
            time.sleep(2.0)

    cm_total = np.zeros((C, C), dtype=np.float64)
    for c, r in enumerate(res.results):
        out = r["cm"].astype(np.float64)
        acc_p, acc_a = out[:, 0:C], out[:, C : 2 * C]
        cm_total += acc_p + (supports_act[c][:, None] - acc_a)
    cm_total[:, 0] -= pad_corr
    diag = np.diagonal(cm_total)
    precision = diag / (cm_total.sum(axis=1) + EPS)
    recall = diag / (cm_total.sum(axis=0) + EPS)
    f1 = 2.0 * precision * recall / (precision + recall + EPS)
    return np.float32(f1.mean()), res


def kernel(y_pred, y_true):
    out, _ = _run(y_pred, y_true, trace=False)
    return out


# revision 14
# speedup vs baseline: 1.7076x; 1.1216x over previous
"""F1-score (macro) kernel for Trainium2, 8 NeuronCores.

Key trick: the confusion matrix is invariant under row permutation, so the
host reorders rows so that most device chunks [128 rows, 128 classes] hold
exactly one row of each true class at partition p = class.  For those chunks
the true-label one-hot stationary is the IDENTITY (a constant in SBUF) -- no
per-chunk oht build at all.  Rows that don't fit the per-class quota go to 32
"regular" chunks with an on-device oht build; missing slots become pad rows
(y_pred = e0, so pred=0) whose exact contribution is subtracted host-side.

Device layout per core: 125056 rows = 128 partitions x 977 j-columns,
row(p, j) = p*977 + j.  16 tiles (15x61 + 1x62 j-cols).  Per tile:
  - SWDGE DMA casts y_pred f32 -> bf16 on the fly (HBM reads stay f32)
  - DVE: rowmax via 7-stage tensor_tensor max tree (2x bf16 mode)
  - ohp[:,j,:]: DVE is_equal(xh, mh) for ~33 chunks -> acc_p,
                ACT sign(mh - xh) (anti form) for 28 chunks -> acc_a
  - PE: acc += stationary^T @ ohp, stationary = identity or built oht
Host: cm = sum_cores [acc_p + (support_act - acc_a)] - pad corrections.

bf16 tie semantics (multi-hot on exact bf16 ties) verified on harness data:
rel err 7.7e-4 << 2e-2.
"""

import sys
import time

if "/opt/trn_rl_repo" not in sys.path:
    sys.path.insert(0, "/opt/trn_rl_repo")

import numpy as np

import concourse.bacc as bacc
import concourse.mybir as mybir
import concourse.tile as tile
from concourse import bass_utils

C = 128
N = 1_000_000
NCORES = 8
R = N // NCORES          # 125000 real rows per core
J = 977                  # j-columns per partition
RD = 128 * J             # 125056 device rows per core
TKS = [61] * 15 + [62]   # j-columns per tile
REG_LOCALS = (29, 60)    # j-locals per tile with on-device oht build
ACT_LOCALS = frozenset(range(27, 61))  # ohp chunks computed on ACT
K_ID = J - 16 * len(REG_LOCALS)        # 945 identity chunks per core
EPS = 1e-12

_J0S = np.cumsum([0] + TKS[:-1]).tolist()
REG_JS = sorted(j0 + l for j0 in _J0S for l in REG_LOCALS)
IDENT_JS = sorted(set(range(J)) - set(REG_JS))
ACT_JS = sorted(
    j0 + l for i, j0 in enumerate(_J0S) for l in range(TKS[i]) if l in ACT_LOCALS
)

_CACHE = {}


def _build():
    f32 = mybir.dt.float32
    bf16 = mybir.dt.bfloat16
    Alu = mybir.AluOpType
    Act = mybir.ActivationFunctionType

    nc = bacc.Bacc("TRN2", target_bir_lowering=False, debug=False,
                   num_devices=NCORES)
    yp = nc.dram_tensor("yp", [RD, C], f32, kind="ExternalInput")
    yt = nc.dram_tensor("yt", [128 * len(REG_JS)], f32, kind="ExternalInput")
    cm = nc.dram_tensor("cm", [C, 2 * C], f32, kind="ExternalOutput")

    grid = yp.ap().rearrange("(p j) c -> p j c", p=128)

    with tile.TileContext(nc) as tc:
        with (
            tc.tile_pool(name="const", bufs=1) as cpool,
            tc.tile_pool(name="xin", bufs=4) as xpool,
            tc.tile_pool(name="tree", bufs=1) as tpool,
            tc.tile_pool(name="oh", bufs=3) as ohpool,
            tc.tile_pool(name="small", bufs=3) as spool,
            tc.tile_pool(name="psum", bufs=1, space="PSUM") as psum,
        ):
            iota_i = cpool.tile([128, C], mybir.dt.int32)
            nc.gpsimd.iota(iota_i[:], pattern=[[1, C]], base=0,
                           channel_multiplier=0)
            iota_h = cpool.tile([128, C], bf16)
            nc.vector.tensor_copy(iota_h[:], iota_i[:])
            # per-partition index column p -> identity = (iota == p)
            pcol_i = cpool.tile([128, 1], mybir.dt.int32)
            nc.gpsimd.iota(pcol_i[:], pattern=[[0, 1]], base=0,
                           channel_multiplier=1)
            pcol = cpool.tile([128, 1], f32)
            nc.vector.tensor_copy(pcol[:], pcol_i[:])
            ident = cpool.tile([128, C], bf16)
            nc.vector.tensor_scalar(
                ident[:], iota_h[:], pcol[:], None, op0=Alu.is_equal
            )

            # true labels for the regular chunks only: [128, 32]
            t_reg = cpool.tile([128, len(REG_JS)], f32)
            nc.sync.dma_start(
                t_reg[:], yt.ap().rearrange("(p k) -> p k", p=128)
            )
            reg_idx = {j: k for k, j in enumerate(REG_JS)}

            acc_p = psum.tile([C, C], f32)
            acc_a = psum.tile([C, C], f32)
            state = {"p": False, "a": False}

            def emit_tile(i):
                j0, tk = _J0S[i], TKS[i]
                xh = xpool.tile([128, tk, C], bf16, tag="xh")
                nc.gpsimd.dma_start(xh[:], grid[:, j0 : j0 + tk, :])

                # oht builds for this tile's regular chunks (DMA-independent)
                ohts = {}
                for l in REG_LOCALS:
                    if l >= tk:
                        continue
                    o = ohpool.tile([128, C], bf16, tag=f"oht{l}")
                    k = reg_idx[j0 + l]
                    nc.vector.tensor_scalar(
                        o[:], iota_h[:], t_reg[:, k : k + 1], None,
                        op0=Alu.is_equal,
                    )
                    ohts[l] = o

                # rowmax tree: 64->32->16->8->4->2->1 (bf16 2x TT stages)
                m1 = tpool.tile([128, tk, 64], bf16, tag="m1")
                nc.vector.tensor_tensor(
                    m1[:], xh[:, :, 0:64], xh[:, :, 64:128], op=Alu.max
                )
                m2 = tpool.tile([128, tk, 32], bf16, tag="m2")
                nc.vector.tensor_tensor(
                    m2[:], m1[:, :, 0:32], m1[:, :, 32:64], op=Alu.max
                )
                m3 = tpool.tile([128, tk, 16], bf16, tag="m3")
                nc.vector.tensor_tensor(
                    m3[:], m2[:, :, 0:16], m2[:, :, 16:32], op=Alu.max
                )
                m4 = tpool.tile([128, tk, 8], bf16, tag="m4")
                nc.vector.tensor_tensor(
                    m4[:], m3[:, :, 0:8], m3[:, :, 8:16], op=Alu.max
                )
                m5 = tpool.tile([128, tk, 4], bf16, tag="m5")
                nc.vector.tensor_tensor(
                    m5[:], m4[:, :, 0:4], m4[:, :, 4:8], op=Alu.max
                )
                m6 = tpool.tile([128, tk, 2], bf16, tag="m6")
                nc.vector.tensor_tensor(
                    m6[:], m5[:, :, 0:2], m5[:, :, 2:4], op=Alu.max
                )
                mh = spool.tile([128, tk], f32, tag="mh")
                nc.vector.tensor_tensor(
                    mh[:, :, None], m6[:, :, 0:1], m6[:, :, 1:2], op=Alu.max
                )

                ohp = ohpool.tile([128, tk, C], bf16, tag="ohp")
                for l in range(tk):
                    stat = ohts.get(l, ident)
                    if l in ACT_LOCALS:
                        nc.scalar.activation(
                            ohp[:, l, :], xh[:, l, :], Act.Sign,
                            bias=mh[:, l : l + 1], scale=-1.0,
                        )
                        acc, key = acc_a, "a"
                    else:
                        nc.vector.tensor_scalar(
                            ohp[:, l, :], xh[:, l, :], mh[:, l : l + 1],
                            None, op0=Alu.is_equal,
                        )
                        acc, key = acc_p, "p"
                    nc.tensor.matmul(
                        acc[:], stat[:], ohp[:, l, :],
                        start=not state[key], stop=False,
                    )
                    state[key] = True

            for i in range(len(TKS)):
                emit_tile(i)

            # close both accumulation groups
            zrow = cpool.tile([1, C], bf16)
            nc.vector.memset(zrow[:], 0.0)
            nc.tensor.matmul(acc_p[:], zrow[:], zrow[:], start=False, stop=True)
            nc.tensor.matmul(acc_a[:], zrow[:], zrow[:], start=False, stop=True)

            out_sb = spool.tile([C, 2 * C], f32, tag="out")
            nc.scalar.copy(out_sb[:, 0:C], acc_p[:])
            nc.scalar.copy(out_sb[:, C : 2 * C], acc_a[:])
            nc.sync.dma_start(cm.ap()[:], out_sb[:])

    nc.compile()
    return nc


def _get_nc():
    if "nc" not in _CACHE:
        _CACHE["nc"] = _build()
    return _CACHE["nc"]


def _layout(yt_i):
    """Assign global rows to device slots.

    Returns per-core: idx [128, J] (global row id, -1 => pad),
    pad_class [128, J] (true class of pad slots, valid where idx < 0),
    and the per-core device true-class grid tcls [128, J].
    """
    idxs, tclss = [], []
    rows_by_class = [np.flatnonzero(yt_i == t) for t in range(C)]
    surplus = []
    per_core_ident = [dict() for _ in range(NCORES)]
    for t in range(C):
        rows_t = rows_by_class[t]
        for c in range(NCORES):
            seg = rows_t[c * K_ID : (c + 1) * K_ID]
            per_core_ident[c][t] = seg
        surplus.append(rows_t[NCORES * K_ID :])
    pool = (
        np.concatenate(surplus)
        if surplus
        else np.zeros(0, dtype=np.int64)
    )
    nreg = 128 * len(REG_JS)
    parts = np.array_split(pool, NCORES)
    ident_js = np.asarray(IDENT_JS)
    reg_js = np.asarray(REG_JS)
    for c in range(NCORES):
        idx = np.full((128, J), -1, dtype=np.int64)
        tcls = np.zeros((128, J), dtype=np.int64)
        for t in range(C):
            seg = per_core_ident[c][t]
            idx[t, ident_js[: len(seg)]] = seg
            tcls[t, ident_js] = t  # pads in ident region keep class t
        part = parts[c]
        take = min(len(part), nreg)
        # fill reg slots p-major: slot k -> (p = k % 128, j = reg_js[k // 128])
        ks = np.arange(take)
        idx[ks % 128, reg_js[ks // 128]] = part[:take]
        tcls[ks % 128, reg_js[ks // 128]] = yt_i[part[:take]]
        # remaining reg slots stay pads with class 0 (tcls already 0)
        idxs.append(idx)
        tclss.append(tcls)
    return idxs, tclss


def _run(y_pred, y_true, trace=False):
    nc = _get_nc()
    y_pred = np.ascontiguousarray(np.asarray(y_pred, dtype=np.float32))
    yt_i = np.asarray(y_true).astype(np.int64)
    idxs, tclss = _layout(yt_i)

    pad_row = np.zeros(C, dtype=np.float32)
    pad_row[0] = 1.0  # pred = 0 for pad rows

    in_maps = []
    supports_act = []
    pad_corr = np.zeros(C, dtype=np.float64)  # pads predict 0: cm[t,0] -= corr
    for c in range(NCORES):
        idx = idxs[c]
        tcls = tclss[c]
        flat = idx.ravel()
        pads = flat < 0
        yp_dev = y_pred[np.where(pads, 0, flat)]
        if pads.any():
            yp_dev[pads] = pad_row
        yp_dev = np.ascontiguousarray(yp_dev)
        t_reg = np.ascontiguousarray(
            tcls[:, REG_JS].astype(np.float32)
        ).ravel()
        in_maps.append({"yp": yp_dev, "yt": t_reg})
        supports_act.append(
            np.bincount(tcls[:, ACT_JS].ravel(), minlength=C).astype(
                np.float64
            )
        )
        pad_corr += np.bincount(
            tcls.ravel()[pads], minlength=C
        ).astype(np.float64)

    res = None
    for attempt in range(3):
        try:
            res = bass_utils.run_bass_kernel_spmd(
                nc, in_maps, core_ids=list(range(NCORES)), trace=trace
            )
            break
        except Exception:
            if attempt == 2:
                raise
            time.sleep(2.0)

    cm_total = np.zeros((C, C), dtype=np.float64)
    for c, r in enumerate(res.results):
        out = r["cm"].astype(np.float64)
        acc_p, acc_a = out[:, 0:C], out[:, C : 2 * C]
        cm_total += acc_p + (supports_act[c][:, None] - acc_a)
    cm_total[:, 0] -= pad_corr
    diag = np.diagonal(cm_total)
    precision = diag / (cm_total.sum(axis=1) + EPS)
    recall = diag / (cm_total.sum(axis=0) + EPS)
    f1 = 2.0 * precision * recall / (precision + recall + EPS)
    return np.float32(f1.mean()), res


def kernel(y_pred, y_true):
    out, _ = _run(y_pred, y_true, trace=False)
    return out
